# revision 15
# baseline (speedup 1.0000x reference)
"""Two-layer GAT (GATConv x2 + classifier) as a Bass/Tile SPMD kernel on 8
Trainium2 NeuronCores.

V2 design (constraints discovered on HW: dma_gather is the only fast gather;
int16 indices -> <=32768-row windows; 256B element grain; matmul operand
partition bases in {0,32,64}):

  * Host: self-loops, sort edges by (dst, class) where class = src < TH.
    Each class packed into dst-segment-aligned tiles (<=128 edges, <=24
    nodes); 2 tiles -> supertile (256 edges, <=48 real compact slots of 64);
    8 supertiles -> DMA group (2048-edge dma_gather). LO-class groups first,
    then HI (uniform counts across cores). Class c gathers from window c of
    the table (windows overlap; threshold chosen so the same class works for
    both the L1 table (global rows) and L2 table (padded AllGather rows)).
  * Stage A (replicated): hx[n] = [h f16 x128 | as f32 x8 | pad] (512B rows)
    + per-shard pass: asadloc[local] = [ad f16 x8 | pad] (256B rows).
  * Edge phase: dma_gather hx[src-window rows]; band dma_gather of asadloc
    (compact dst slots, supertile-pair layout at partition bases {0,64});
    e_d via SegB matmuls (host one-hot [k, edge]); e=lrelu(as+e_d);
    exp -> fp16; msg = h*ex; aggregate via on-device SegC one-hot matmuls
    into compact psum; write [48, 256B f16] slabs to nd[st*48:+48] (static).
  * B2: two nd-row gathers (LO/HI partial) + merge-add; h2=elu(num/den);
    h2x rows [h2p f16 16 | as2 f32 | ad2 f16 @ col 18] -> h2x_sh; AllGather.
  * L2 edge phase: same machinery, 1 head, over h2x_full.
  * D2/E: two nd2-row gathers + merge; emb; logits=emb@Wc; softmax; argmax;
    shard-shaped outputs (static writes); host stitches by node range.
"""
import os
import numpy as np
from concourse import bass, mybir, tile, bacc
from concourse.bass_utils import run_bass_kernel_spmd

f16, f32, i32, i16 = (mybir.dt.float16, mybir.dt.float32, mybir.dt.int32,
                      mybir.dt.int16)
A = mybir.AluOpType
ACT = mybir.ActivationFunctionType
AX = mybir.AxisListType


class Cfg:
    def __init__(self, N=50000, E=1000000, IN_F=128, HID=16, HEADS=8, OUT_F=40,
                 neg_slope=0.2, ncores=8, win_rows=32768):
        self.N, self.E = N, E
        self.IN_F, self.HID, self.HEADS, self.OUT_F = IN_F, HID, HEADS, OUT_F
        self.neg = neg_slope
        self.ncores = ncores
        self.win = win_rows        # max table rows addressable per gather
        self.TILE_E = 128
        self.TPS = 2               # tiles per supertile
        self.KC = 64               # compact slots per supertile (pair base 0/64)
        self.KR = 48               # real compact slots (nd rows per supertile)
        self.NCAP = 24             # node cap per tile
        self.SPG = 8               # supertiles per group
        self.GRP_T = self.TPS * self.SPG   # 16 tiles per group
        self.GRP_E = self.GRP_T * self.TILE_E  # 2048 edges per group
        self.HD = HEADS * HID              # 128
        self.C1 = self.HD + HEADS          # 136 psum cols L1
        self.C2 = HID + 1                  # 17 psum cols L2
        self.HXW = 256                     # hx row f16 words (512B)
        self.NDW = 256                     # nd1 row f16 words
        self.ND2W = 128                    # nd2 row f16 words
        self.H2XW = 128                    # h2x row f16 words
        self.ADW = 128                     # asadloc row f16 words


DROPK = 63  # dump compact slot for pad edges (>= KR, never written out)


# ----------------------------------------------------------------- host prep

def _pack_class(nodes_iter, deg_c, cfg):
    """Greedy pack nodes (with class-degree deg_c) into tiles.
    Returns list of tiles: list of (node, k_slot_base) lists."""
    tiles = []
    cur = []
    cur_e = 0
    for n in nodes_iter:
        d = int(deg_c[n])
        if d == 0:
            continue
        assert d <= cfg.TILE_E
        if cur_e + d > cfg.TILE_E or len(cur) >= cfg.NCAP:
            tiles.append(cur)
            cur = []
            cur_e = 0
        cur.append(n)
        cur_e += d
    if cur:
        tiles.append(cur)
    return tiles


def preprocess(edge_index, cfg):
    N = cfg.N
    P = 128
    src = np.concatenate([np.asarray(edge_index[0], np.int64),
                          np.arange(N, dtype=np.int64)])
    dst = np.concatenate([np.asarray(edge_index[1], np.int64),
                          np.arange(N, dtype=np.int64)])
    Npad = -(-N // P) * P
    cfg.Npad = Npad
    deg = np.bincount(dst, minlength=N)
    assert deg.max() <= cfg.TILE_E and deg.min() >= 1
    cum = np.cumsum(deg)
    Etot = src.shape[0]
    bounds = [0]
    for k in range(1, cfg.ncores):
        bounds.append(int(np.searchsorted(cum, Etot * k / cfg.ncores)))
    bounds.append(N)
    node_lo = np.array(bounds[:-1], np.int64)
    node_hi = np.array(bounds[1:], np.int64)

    S = int((node_hi - node_lo).max())
    batch = 1024 if S > 1024 else 128
    S_pad = -(-S // batch) * batch
    cfg.S_pad = S_pad
    rows2 = cfg.ncores * S_pad
    w2b_off = max(0, rows2 - cfg.win)
    core_of = np.repeat(np.arange(cfg.ncores), node_hi - node_lo)
    pad_row = (core_of * S_pad + np.arange(N) - node_lo[core_of]).astype(np.int64)

    # class threshold: src < TH -> window A of both tables; else window B
    w1b_off = max(0, Npad - cfg.win)
    th_lo = max(w1b_off, int(np.searchsorted(pad_row, w2b_off)))
    th_hi = min(cfg.win, int(np.searchsorted(pad_row, cfg.win)))
    assert th_lo < th_hi, f"no valid class threshold ({th_lo}, {th_hi})"
    TH = (th_lo + th_hi) // 2
    if w1b_off == 0 and w2b_off == 0:
        TH = Npad  # single class
    cls = (src >= TH).astype(np.int64)
    order = np.lexsort((src, dst))  # dst-major; src order groups classes
    src_s = src[order].astype(np.int64)
    dst_s = dst[order].astype(np.int64)
    cls_s = cls[order]
    deg_lo = np.bincount(dst_s[cls_s == 0], minlength=N)
    deg_hi = deg - deg_lo

    core_tiles = []   # per core: (tiles_lo, tiles_hi)
    for k in range(cfg.ncores):
        rng = range(node_lo[k], node_hi[k])
        core_tiles.append((_pack_class(rng, deg_lo, cfg),
                           _pack_class(rng, deg_hi, cfg)))

    def pad_groups(n):
        return -(-max(n, 1) // cfg.GRP_T) * cfg.GRP_T

    TLO = pad_groups(max(len(t[0]) for t in core_tiles))
    THI = pad_groups(max(len(t[1]) for t in core_tiles))
    T = TLO + THI
    nst = T // cfg.TPS
    ngrp = T // cfg.GRP_T
    cfg.ngrpLO = TLO // cfg.GRP_T
    cfg.ngrpHI = THI // cfg.GRP_T
    cfg.ngrp = ngrp
    nd_rows = nst * cfg.KR
    assert nd_rows <= cfg.win, f"nd table {nd_rows} rows exceeds window"
    cfg.nd_rows = nd_rows
    cfg.nst = nst
    # class consistency for L2 windows
    assert pad_row[(np.arange(N) < TH)].max(initial=0) < cfg.win
    if (np.arange(N) >= TH).any() and w2b_off > 0:
        assert pad_row[(np.arange(N) >= TH)].min() >= w2b_off
    cfg.TH, cfg.w1b_off, cfg.w2b_off = int(TH), int(w1b_off), int(w2b_off)

    # edge offsets per (node, class): starts within sorted arrays
    seg_start = np.zeros(N + 1, np.int64)
    seg_start[1:] = cum
    # lo edges of node n: [seg_start[n], seg_start[n]+deg_lo[n]) ; hi after

    cores = []
    for k in range(cfg.ncores):
        tiles_lo, tiles_hi = core_tiles[k]
        all_tiles = (list(tiles_lo) + [[]] * (TLO - len(tiles_lo))
                     + list(tiles_hi) + [[]] * (THI - len(tiles_hi)))
        src1 = np.zeros((T, cfg.TILE_E), np.int64)      # window-rel L1 idx
        src2 = np.zeros((T, cfg.TILE_E), np.int64)      # window-rel L2 idx
        dstcomp = np.full((T, cfg.TILE_E), DROPK, np.int64)
        bandloc = np.zeros((nst, cfg.KC), np.int64)     # local node row per slot
        nreal = np.zeros(nst, np.int64)
        # node -> (ndrow) per class
        ndrow = np.zeros((2, S_pad), np.int64)          # filled with zrow later
        ndrow_set = np.zeros((2, S_pad), bool)

        for t, nodes in enumerate(all_tiles):
            is_hi = t >= TLO
            st = t // cfg.TPS
            koff = int(nreal[st])
            ecur = 0
            for n in nodes:
                d = int((deg_hi if is_hi else deg_lo)[n])
                e0 = int(seg_start[n] + (deg_lo[n] if is_hi else 0))
                sl = slice(e0, e0 + d)
                s_glob = src_s[sl]
                woff1 = cfg.w1b_off if is_hi else 0
                woff2 = cfg.w2b_off if is_hi else 0
                src1[t, ecur:ecur + d] = s_glob - woff1
                src2[t, ecur:ecur + d] = pad_row[s_glob] - woff2
                kslot = koff
                dstcomp[t, ecur:ecur + d] = kslot
                bandloc[st, kslot] = n - node_lo[k]
                ndrow[int(is_hi), n - node_lo[k]] = st * cfg.KR + kslot
                ndrow_set[int(is_hi), n - node_lo[k]] = True
                koff += 1
                ecur += d
            nreal[st] = koff
            assert koff <= cfg.KR
        assert (src1 >= 0).all() and (src1 < cfg.win).all()
        assert (src2 >= 0).all() and (src2 < cfg.win).all()

        # zero row: first supertile with nreal < KR
        zcand = np.nonzero(nreal < cfg.KR)[0]
        assert len(zcand) > 0
        zrow = int(zcand[0] * cfg.KR + nreal[zcand[0]])
        ndrow[~ndrow_set] = zrow

        segb = np.zeros((T, cfg.KC, cfg.TILE_E), np.float16)
        kk = np.arange(cfg.KC)[None, :, None]
        segb[:] = (dstcomp[:, None, :] == kk)

        cores.append(dict(src1=src1, src2=src2, dstcomp=dstcomp,
                          bandloc=bandloc, nreal=nreal, ndrow=ndrow,
                          segb=segb, node_lo=int(node_lo[k]),
                          node_hi=int(node_hi[k])))
    return dict(cores=cores, T=T, node_lo=node_lo, node_hi=node_hi,
                pad_row=pad_row, batch=batch)


def _wrap16(vals):
    """vals [n] -> int16 [128, n//16] in the dma_gather wrapped layout."""
    n = vals.shape[-1]
    assert n % 16 == 0
    lead = vals.shape[:-1]
    w = np.zeros(lead + (128, n // 16), np.int16)
    v = vals.reshape(lead + (n // 16, 16))
    w[..., :16, :] = np.swapaxes(v, -1, -2)
    for r in range(1, 8):
        w[..., r * 16:(r + 1) * 16, :] = w[..., :16, :]
    return w


def group_layout(cfg, pp):
    out = []
    GT, TE, KC = cfg.GRP_T, cfg.TILE_E, cfg.KC
    ngrp = cfg.ngrp
    for c in pp["cores"]:
        d = {}
        d["gsrc1"] = _wrap16(c["src1"].reshape(ngrp, GT * TE))
        d["gsrc2"] = _wrap16(c["src2"].reshape(ngrp, GT * TE))
        # band: j = pair*128 + (s%2)*64 + kc -> local row
        bl = c["bandloc"].reshape(ngrp, cfg.SPG * KC)
        d["gband"] = _wrap16(bl)
        d["dstcomp"] = (c["dstcomp"].astype(np.float16)
                        .reshape(ngrp, GT, TE).transpose(0, 2, 1).copy())
        # segb sbuf layout [128, GT, 128]: partitions (s%2)*64 + kc for tile t
        sb = np.zeros((ngrp, 128, GT, TE), np.float16)
        segb = c["segb"].reshape(ngrp, GT, KC, TE)
        for t in range(GT):
            half = (t // cfg.TPS) % 2
            sb[:, half * 64:(half + 1) * 64, t, :] = segb[:, t]
        d["gsegb"] = sb
        # nd row gathers for B2/D2E: [nbat, 128, batch//16] for each class
        nb = cfg.S_pad // pp["batch"]
        d["gndlo"] = _wrap16(c["ndrow"][0].reshape(nb, pp["batch"]))
        d["gndhi"] = _wrap16(c["ndrow"][1].reshape(nb, pp["batch"]))
        out.append(d)
    return out


# ------------------------------------------------------------ device program

def emit(tc, nc, cfg, t):
    P = 128
    HD, H, HID = cfg.HD, cfg.HEADS, cfg.HID
    NTA = cfg.Npad // P
    NTS = cfg.S_pad // P
    nbat = cfg.S_pad // cfg.batch
    bat_t = cfg.batch // P   # node tiles per gather batch

    from contextlib import ExitStack
    stk = ExitStack()
    cp = stk.enter_context(tc.tile_pool(name="consts", bufs=1))
    w1x_sb = cp.tile([P, HD + 2 * H], f16)       # [W1 | Ws | Wd] fp16
    nc.sync.dma_start(out=w1x_sb[:], in_=t["W1x16"].ap())
    w2x_sb = cp.tile([P, HID + 2], f16)
    nc.sync.dma_start(out=w2x_sb[:], in_=t["W2x16"].ap())
    wcx_sb = cp.tile([HID, cfg.OUT_F], f16)
    nc.sync.dma_start(out=wcx_sb[:], in_=t["Wcx16"].ap())
    iotaK = cp.tile([P, cfg.KC], f16)
    nc.sync.dma_start(out=iotaK[:], in_=t["iotaK16"].ap())
    ident = cp.tile([P, P], f32)
    nc.sync.dma_start(out=ident[:], in_=t["ident32"].ap())
    inv40 = cp.tile([P, cfg.OUT_F], f32)
    nc.sync.dma_start(out=inv40[:], in_=t["inv40"].ap())

    # ---------------- Stage A: hx rows [h f16 128 | as f32 8 | pad]
    with tc.tile_pool(name="sa", bufs=4) as pa, \
         tc.tile_pool(name="sa_ps", bufs=4, space="PSUM") as pap:
        for nt in range(NTA):
            xt = pa.tile([P, P], f16)
            nc.sync.dma_start(out=xt[:], in_=t["xT16"].ap()[:, nt * P:(nt + 1) * P])
            ps = pap.tile([P, HD + 2 * H], f32)
            nc.tensor.matmul(out=ps[:], lhsT=xt[:], rhs=w1x_sb[:],
                             start=True, stop=True)
            hxt = pa.tile([P, cfg.HXW], f16)
            nc.vector.memset(hxt[:], 0.0)
            nc.scalar.copy(out=hxt[:, 0:HD], in_=ps[:, 0:HD])
            hxf = hxt[:].bitcast(f32)
            nc.vector.tensor_copy(out=hxf[:, 64:64 + H], in_=ps[:, HD:HD + H])
            nc.sync.dma_start(out=t["hx"].ap()[nt * P:(nt + 1) * P, :], in_=hxt[:])
        # per-shard: asadloc rows [ad f16 8 | pad]
        for ntb in range(NTS):
            xt = pa.tile([P, P], f16, tag="xtl")
            nc.sync.dma_start(out=xt[:],
                              in_=t["xTloc"].ap()[:, ntb * P:(ntb + 1) * P])
            ps2 = pap.tile([P, H], f32, tag="psl")
            nc.tensor.matmul(out=ps2[:], lhsT=xt[:],
                             rhs=w1x_sb[:, HD + H:HD + 2 * H],
                             start=True, stop=True)
            adt = pa.tile([P, cfg.ADW], f16, tag="adt")
            nc.vector.memset(adt[:], 0.0)
            nc.scalar.copy(out=adt[:, 0:H], in_=ps2[:])
            nc.sync.dma_start(out=t["asadloc"].ap()[ntb * P:(ntb + 1) * P, :],
                              in_=adt[:])

    # ---------------- L1 / L2 edge phases
    ph = int(os.environ.get("GAT_PHASES", "5"))
    if ph >= 1:
        edge_phase(tc, nc, cfg, t, 1)
    if ph >= 2:
        b2_phase(tc, nc, cfg, t, ident, w2x_sb, nbat, bat_t)
    if ph >= 3:
        nc.gpsimd.collective_compute(
            "AllGather", A.bypass, replica_groups=[list(range(cfg.ncores))],
            ins=[t["h2x_sh"].ap()], outs=[t["h2x_full"].ap()])
    if ph >= 4:
        edge_phase(tc, nc, cfg, t, 2)
    if ph >= 5:
        d2e_phase(tc, nc, cfg, t, ident, wcx_sb, inv40, nbat, bat_t)
    stk.close()


def edge_phase(tc, nc, cfg, t, layer):
    P = 128
    GT, TE, KC, SPG = cfg.GRP_T, cfg.TILE_E, cfg.KC, cfg.SPG
    if layer == 1:
        H, FH, CC = cfg.HEADS, cfg.HD, cfg.C1
        src_t, table, ndt = t["gsrc1"], t["hx"], t["nd1"]
        twords, ndw = cfg.HXW, cfg.NDW
        band_tab, ad_f32_off, ad_is_f32 = t["asadloc"], 0, False  # ad f16 cols 0:8
        adw = cfg.ADW
    else:
        H, FH, CC = 1, cfg.HID, cfg.C2
        src_t, table, ndt = t["gsrc2"], t["h2x_full"], t["nd2"]
        twords, ndw = cfg.H2XW, cfg.ND2W
        band_tab, adw = t["h2x_sh"], cfg.H2XW  # ad2 f16 at col 18
    w1off = cfg.w1b_off if layer == 1 else cfg.w2b_off
    iotaK = t["_iotaK"]

    with tc.tile_pool(name=f"l{layer}", bufs=2) as pl, \
         tc.tile_pool(name=f"l{layer}r", bufs=3) as pr, \
         tc.tile_pool(name=f"l{layer}ps", bufs=3, space="PSUM") as plp:
        for g in range(cfg.ngrp):
            is_hi = g >= cfg.ngrpLO
            woff = w1off if is_hi else 0
            six = pl.tile([P, GT * TE // 16], i16)
            nc.sync.dma_start(out=six[:], in_=src_t.ap()[g])
            bix = pl.tile([P, SPG * KC // 16], i16)
            nc.sync.dma_start(out=bix[:], in_=t["gband"].ap()[g])
            dcmp = pl.tile([P, GT], f16)
            nc.sync.dma_start(out=dcmp[:], in_=t["dstcomp"].ap()[g])
            segb = pl.tile([P, GT, TE], f16)
            nc.sync.dma_start(out=segb[:].rearrange("p a b -> p (a b)"),
                              in_=t["gsegb"].ap()[g].rearrange("p a b -> p (a b)"))
            # edge gather
            hxg = pl.tile([P, GT, twords], f16)
            tab_rows = table.shape[0]
            win_ap = table.ap()[woff:min(woff + cfg.win, tab_rows), :]
            nh = GT * TE // 1024  # dma_gather caps at 1024 idxs
            for h8 in range(nh):
                tpg = 1024 // TE
                nc.gpsimd.dma_gather(
                    out_ap=hxg[:, h8 * tpg:(h8 + 1) * tpg, :], in_ap=win_ap,
                    idxs_ap=six[:, h8 * 64:(h8 + 1) * 64],
                    num_idxs=1024, num_idxs_reg=1024, elem_size=twords)
            # band gather (compact ad rows), pair layout
            adg = pl.tile([P, SPG * KC // 128, adw], f16)
            nc.gpsimd.dma_gather(
                out_ap=adg[:], in_ap=band_tab.ap(),
                idxs_ap=bix[:], num_idxs=SPG * KC, num_idxs_reg=SPG * KC,
                elem_size=adw)
            # SegC one-hot [128, GT, KC]
            segc = pl.tile([P, GT, KC], f16)
            nc.vector.tensor_tensor(
                out=segc[:],
                in0=iotaK[:].rearrange("p (o r) -> p o r", o=1).to_broadcast([P, GT, KC]),
                in1=dcmp[:].rearrange("p (a o) -> p a o", o=1).to_broadcast([P, GT, KC]),
                op=A.is_equal)
            for sp in range(SPG // 2):      # supertile pairs
                ndp = plp.tile([P, CC], f32)
                for sh in range(2):         # supertile s = sp*2+sh
                    s = sp * 2 + sh
                    # e = as + e_d ; lrelu ; exp(fp16)
                    e0 = pr.tile([P, cfg.TPS, H], f32, tag="e0")
                    for tt in range(cfg.TPS):
                        ti = s * cfg.TPS + tt
                        if layer == 1:
                            rhs_ad = adg[sh * 64:(sh + 1) * 64, sp, 0:H]
                        else:
                            rhs_ad = adg[sh * 64:(sh + 1) * 64, sp, 18:18 + H]
                        edp = plp.tile([P, H], f32, tag="edp")
                        nc.tensor.matmul(
                            out=edp[:],
                            lhsT=segb[sh * 64:(sh + 1) * 64, ti, :],
                            rhs=rhs_ad, start=True, stop=True,
                            tile_position=(sh * 64, 0))
                        if layer == 1:
                            as_ap = hxg[:].bitcast(f32)[:, ti, 64:64 + H]
                        else:
                            as_ap = hxg[:].bitcast(f32)[:, ti, 8:8 + H]
                        nc.vector.tensor_tensor(out=e0[:, tt, :], in0=as_ap,
                                                in1=edp[:], op=A.add)
                    es = pr.tile([P, cfg.TPS, H], f32, tag="es")
                    nc.vector.tensor_scalar(out=es[:], in0=e0[:], scalar1=cfg.neg,
                                            scalar2=None, op0=A.mult)
                    nc.vector.tensor_tensor(out=e0[:], in0=e0[:], in1=es[:],
                                            op=A.max)
                    rhst = pr.tile([P, cfg.TPS, CC], f16, tag="rhst")
                    nc.scalar.activation(out=rhst[:, :, FH:FH + H], in_=e0[:],
                                         func=ACT.Exp)
                    # msg = h * ex
                    nc.vector.tensor_tensor(
                        out=rhst[:, :, 0:FH].rearrange("p a (h c) -> p a h c", h=H),
                        in0=hxg[:, s * cfg.TPS:(s + 1) * cfg.TPS, 0:FH].rearrange(
                            "p a (h c) -> p a h c", h=H),
                        in1=rhst[:, :, FH:FH + H].rearrange(
                            "p a (h o) -> p a h o", o=1).to_broadcast(
                                [P, cfg.TPS, H, cfg.HID]),
                        op=A.mult)
                    # aggregate into compact psum rows [sh*64, +64)
                    for tt in range(cfg.TPS):
                        ti = s * cfg.TPS + tt
                        nc.tensor.matmul(
                            out=ndp[sh * 64:(sh + 1) * 64, :],
                            lhsT=segc[:, ti, :], rhs=rhst[:, tt, :],
                            start=(tt == 0), stop=(tt == cfg.TPS - 1),
                            tile_position=(0, sh * 64))
                # write both supertiles' nd slabs
                for sh in range(2):
                    s = sp * 2 + sh
                    st = g * SPG + s
                    ndsb = pr.tile([cfg.KR, ndw], f16, tag="ndsb")
                    nc.vector.memset(ndsb[:], 0.0)
                    nc.scalar.copy(out=ndsb[:, 0:FH],
                                   in_=ndp[sh * 64:sh * 64 + cfg.KR, 0:FH])
                    ndf = ndsb[:].bitcast(f32)
                    nc.vector.tensor_copy(
                        out=ndf[:, ndw // 4:ndw // 4 + H],
                        in_=ndp[sh * 64:sh * 64 + cfg.KR, FH:FH + H])
                    nc.sync.dma_start(
                        out=ndt.ap()[st * cfg.KR:(st + 1) * cfg.KR, :],
                        in_=ndsb[:])


def b2_phase(tc, nc, cfg, t, ident, w2x_sb, nbat, bat_t):
    """num/den merge -> h2 = elu(out1) -> h2x rows + AllGather input."""
    P = 128
    H, HID, HD = cfg.HEADS, cfg.HID, cfg.HD
    ndw = cfg.NDW
    with tc.tile_pool(name="b2", bufs=2) as pb, \
         tc.tile_pool(name="b2ps", bufs=3, space="PSUM") as pbp:
        for b in range(nbat):
            ilo = pb.tile([P, cfg.batch // 16], i16)
            nc.sync.dma_start(out=ilo[:], in_=t["gndlo"].ap()[b])
            ihi = pb.tile([P, cfg.batch // 16], i16)
            nc.sync.dma_start(out=ihi[:], in_=t["gndhi"].ap()[b])
            glo = pb.tile([P, bat_t, ndw], f16)
            nc.gpsimd.dma_gather(out_ap=glo[:], in_ap=t["nd1"].ap(),
                                 idxs_ap=ilo[:], num_idxs=cfg.batch,
                                 num_idxs_reg=cfg.batch, elem_size=ndw)
            ghi = pb.tile([P, bat_t, ndw], f16)
            nc.gpsimd.dma_gather(out_ap=ghi[:], in_ap=t["nd1"].ap(),
                                 idxs_ap=ihi[:], num_idxs=cfg.batch,
                                 num_idxs_reg=cfg.batch, elem_size=ndw)
            for bt in range(bat_t):
                num = pb.tile([P, HD], f16, tag="num")
                nc.vector.tensor_tensor(out=num[:], in0=glo[:, bt, 0:HD],
                                        in1=ghi[:, bt, 0:HD], op=A.add)
                den = pb.tile([P, H], f32, tag="den")
                nc.vector.tensor_tensor(
                    out=den[:],
                    in0=glo[:].bitcast(f32)[:, bt, ndw // 4:ndw // 4 + H],
                    in1=ghi[:].bitcast(f32)[:, bt, ndw // 4:ndw // 4 + H],
                    op=A.add)
                rden = pb.tile([P, H], f32, tag="rden")
                nc.vector.reciprocal(out=rden[:], in_=den[:])
                out1 = pb.tile([P, H, HID], f32, tag="out1")
                nc.vector.tensor_tensor(
                    out=out1[:],
                    in0=num[:].rearrange("p (h c) -> p h c", h=H),
                    in1=rden[:].rearrange("p (h o) -> p h o", o=1).to_broadcast(
                        [P, H, HID]),
                    op=A.mult)
                o1f = out1[:].rearrange("p h c -> p (h c)")
                # elu = relu(x) + exp(min(x,0)) - 1
                tmin = pb.tile([P, HD], f32, tag="tmin")
                nc.vector.tensor_scalar(out=tmin[:], in0=o1f, scalar1=0.0,
                                        scalar2=None, op0=A.min)
                texp = pb.tile([P, HD], f32, tag="texp")
                nc.scalar.activation(out=texp[:], in_=tmin[:], func=ACT.Exp)
                h2 = pb.tile([P, HD], f32, tag="h2")
                nc.vector.tensor_scalar(out=h2[:], in0=o1f, scalar1=0.0,
                                        scalar2=None, op0=A.max)
                nc.vector.tensor_tensor(out=h2[:], in0=h2[:], in1=texp[:], op=A.add)
                nc.vector.tensor_scalar(out=h2[:], in0=h2[:], scalar1=1.0,
                                        scalar2=None, op0=A.subtract)
                tp = pbp.tile([P, P], f32, tag="tp")
                nc.tensor.transpose(out=tp[:], in_=h2[:], identity=ident[:])
                h2t = pb.tile([P, P], f16, tag="h2t")
                nc.scalar.copy(out=h2t[:], in_=tp[:])
                ps2 = pbp.tile([P, HID + 2], f32, tag="ps2")
                nc.tensor.matmul(out=ps2[:], lhsT=h2t[:], rhs=w2x_sb[:],
                                 start=True, stop=True)
                h2x = pb.tile([P, cfg.H2XW], f16, tag="h2x")
                nc.vector.memset(h2x[:], 0.0)
                nc.scalar.copy(out=h2x[:, 0:HID], in_=ps2[:, 0:HID])
                h2xf = h2x[:].bitcast(f32)
                nc.vector.tensor_copy(out=h2xf[:, 8:9], in_=ps2[:, HID:HID + 1])
                nc.vector.tensor_copy(out=h2x[:, 18:19], in_=ps2[:, HID + 1:HID + 2])
                r0 = (b * bat_t + bt) * P
                nc.sync.dma_start(out=t["h2x_sh"].ap()[r0:r0 + P, :], in_=h2x[:])


def d2e_phase(tc, nc, cfg, t, ident, wcx_sb, inv40, nbat, bat_t):
    P = 128
    HID, OF = cfg.HID, cfg.OUT_F
    ndw = cfg.ND2W
    with tc.tile_pool(name="e2", bufs=2) as pe, \
         tc.tile_pool(name="e2ps", bufs=3, space="PSUM") as pep:
        for b in range(nbat):
            ilo = pe.tile([P, cfg.batch // 16], i16)
            nc.sync.dma_start(out=ilo[:], in_=t["gndlo"].ap()[b])
            ihi = pe.tile([P, cfg.batch // 16], i16)
            nc.sync.dma_start(out=ihi[:], in_=t["gndhi"].ap()[b])
            glo = pe.tile([P, bat_t, ndw], f16)
            nc.gpsimd.dma_gather(out_ap=glo[:], in_ap=t["nd2"].ap(),
                                 idxs_ap=ilo[:], num_idxs=cfg.batch,
                                 num_idxs_reg=cfg.batch, elem_size=ndw)
            ghi = pe.tile([P, bat_t, ndw], f16)
            nc.gpsimd.dma_gather(out_ap=ghi[:], in_ap=t["nd2"].ap(),
                                 idxs_ap=ihi[:], num_idxs=cfg.batch,
                                 num_idxs_reg=cfg.batch, elem_size=ndw)
            for bt in range(bat_t):
                r0 = (b * bat_t + bt) * P
                num = pe.tile([P, HID], f16, tag="num")
                nc.vector.tensor_tensor(out=num[:], in0=glo[:, bt, 0:HID],
                                        in1=ghi[:, bt, 0:HID], op=A.add)
                den = pe.tile([P, 1], f32, tag="den")
                nc.vector.tensor_tensor(
                    out=den[:],
                    in0=glo[:].bitcast(f32)[:, bt, ndw // 4:ndw // 4 + 1],
                    in1=ghi[:].bitcast(f32)[:, bt, ndw // 4:ndw // 4 + 1],
                    op=A.add)
                rden = pe.tile([P, 1], f32, tag="rden")
                nc.vector.reciprocal(out=rden[:], in_=den[:])
                emb = pe.tile([P, HID], f32, tag="emb")
                nc.vector.tensor_scalar(out=emb[:], in0=num[:],
                                        scalar1=rden[:, 0:1], scalar2=None,
                                        op0=A.mult)
                nc.sync.dma_start(out=t["emb_sh"].ap()[r0:r0 + P, :], in_=emb[:])
                embp = pe.tile([P, P], f32, tag="embp")
                nc.vector.memset(embp[:], 0.0)
                nc.vector.tensor_copy(out=embp[:, 0:HID], in_=emb[:])
                tp = pep.tile([P, P], f32, tag="tp")
                nc.tensor.transpose(out=tp[:], in_=embp[:], identity=ident[:])
                embt = pe.tile([HID, P], f16, tag="embt")
                nc.scalar.copy(out=embt[:], in_=tp[:HID, :])
                lgp = pep.tile([P, OF], f32, tag="lgp")
                nc.tensor.matmul(out=lgp[:], lhsT=embt[:], rhs=wcx_sb[:],
                                 start=True, stop=True)
                lg = pe.tile([P, OF], f32, tag="lg")
                nc.vector.tensor_copy(out=lg[:], in_=lgp[:])
                nc.sync.dma_start(out=t["logits_sh"].ap()[r0:r0 + P, :], in_=lg[:])
                nmx = pe.tile([P, 1], f32, tag="nmx")
                nc.vector.tensor_reduce(out=nmx[:], in_=lg[:], axis=AX.X,
                                        op=A.max, negate=True)
                ex40 = pe.tile([P, OF], f32, tag="ex40")
                nc.scalar.activation(out=ex40[:], in_=lg[:], func=ACT.Exp,
                                     bias=nmx[:, 0:1])
                sm = pe.tile([P, 1], f32, tag="sm")
                nc.vector.tensor_reduce(out=sm[:], in_=ex40[:], axis=AX.X, op=A.add)
                rs = pe.tile([P, 1], f32, tag="rs")
                nc.vector.reciprocal(out=rs[:], in_=sm[:])
                soft = pe.tile([P, OF], f32, tag="soft")
                nc.vector.tensor_scalar(out=soft[:], in0=ex40[:],
                                        scalar1=rs[:, 0:1], scalar2=None,
                                        op0=A.mult)
                nc.sync.dma_start(out=t["soft_sh"].ap()[r0:r0 + P, :], in_=soft[:])
                mx = pe.tile([P, 1], f32, tag="mx")
                nc.vector.tensor_scalar(out=mx[:], in0=nmx[:], scalar1=-1.0,
                                        scalar2=None, op0=A.mult)
                eq = pe.tile([P, OF], f32, tag="eq")
                nc.vector.tensor_scalar(out=eq[:], in0=lg[:], scalar1=mx[:, 0:1],
                                        scalar2=None, op0=A.is_equal)
                val = pe.tile([P, OF], f32, tag="val")
                nc.vector.tensor_tensor(out=val[:], in0=eq[:], in1=inv40[:],
                                        op=A.mult)
                am = pe.tile([P, 1], f32, tag="am")
                nc.vector.tensor_reduce(out=am[:], in_=val[:], axis=AX.X, op=A.min)
                hardf = pe.tile([P, 1], f32, tag="hardf")
                nc.vector.tensor_scalar(out=hardf[:], in0=am[:], scalar1=1024.0,
                                        scalar2=None, op0=A.add)
                hardi = pe.tile([P, 1], i32, tag="hardi")
                nc.vector.tensor_copy(out=hardi[:], in_=hardf[:])
                nc.sync.dma_start(out=t["hard_sh"].ap()[r0:r0 + P, :], in_=hardi[:])


# ------------------------------------------------------------------- runner

def build_inputs(inputs, cfg, pp, gl):
    x = np.asarray(inputs["x"], np.float32)
    W1 = np.asarray(inputs["W1"], np.float32)
    a1s = np.asarray(inputs["a1_src"], np.float32)
    a1d = np.asarray(inputs["a1_dst"], np.float32)
    W2 = np.asarray(inputs["W2"], np.float32)
    a2s = np.asarray(inputs["a2_src"], np.float32)
    a2d = np.asarray(inputs["a2_dst"], np.float32)
    Wc = np.asarray(inputs["Wc"], np.float32)
    H, HID = cfg.HEADS, cfg.HID

    A1s = np.zeros((cfg.HD, H), np.float32)
    A1d = np.zeros((cfg.HD, H), np.float32)
    for h in range(H):
        A1s[h * HID:(h + 1) * HID, h] = a1s[h]
        A1d[h * HID:(h + 1) * HID, h] = a1d[h]
    W1x = np.concatenate([W1, W1 @ A1s, W1 @ A1d], 1).astype(np.float16)
    W2x = np.concatenate([W2, W2 @ a2s.T, W2 @ a2d.T], 1).astype(np.float16)
    Wcx = Wc.astype(np.float16)

    xT = np.zeros((128, cfg.Npad), np.float16)
    xT[:, :cfg.N] = x.T.astype(np.float16)
    iotaK = np.tile(np.arange(cfg.KC, dtype=np.float16), (128, 1))
    ident32 = np.eye(128, dtype=np.float32)
    inv40 = np.tile((np.arange(cfg.OUT_F) - 1024).astype(np.float32), (128, 1))
    common = dict(W1x16=W1x, W2x16=W2x, Wcx16=Wcx, xT16=xT,
                  iotaK16=iotaK, ident32=ident32, inv40=inv40)

    in_maps = []
    for k in range(cfg.ncores):
        c, g = pp["cores"][k], gl[k]
        lo, hi = c["node_lo"], c["node_hi"]
        xTloc = np.zeros((128, cfg.S_pad), np.float16)
        xTloc[:, :hi - lo] = x[lo:hi].T.astype(np.float16)
        m = dict(common)
        m.update(xTloc=xTloc, gsrc1=g["gsrc1"], gsrc2=g["gsrc2"],
                 gband=g["gband"], dstcomp=g["dstcomp"], gsegb=g["gsegb"],
                 gndlo=g["gndlo"], gndhi=g["gndhi"])
        in_maps.append(m)
    return in_maps


def build_program(cfg):
    nc = bacc.Bacc("TRN2", target_bir_lowering=False, debug=False,
                   num_devices=cfg.ncores)
    P = 128
    t = {}

    def inp(name, shape, dt):
        t[name] = nc.dram_tensor(name, list(shape), dt, kind="ExternalInput")

    def outp(name, shape, dt):
        t[name] = nc.dram_tensor(name, list(shape), dt, kind="ExternalOutput")

    def intern(name, shape, dt, addr_space="Local"):
        t[name] = nc.dram_tensor(name, list(shape), dt, kind="Internal",
                                 addr_space=addr_space)

    GT, TE = cfg.GRP_T, cfg.TILE_E
    inp("xT16", (P, cfg.Npad), f16)
    inp("xTloc", (P, cfg.S_pad), f16)
    inp("W1x16", (P, cfg.HD + 2 * cfg.HEADS), f16)
    inp("W2x16", (P, cfg.HID + 2), f16)
    inp("Wcx16", (cfg.HID, cfg.OUT_F), f16)
    inp("iotaK16", (P, cfg.KC), f16)
    inp("ident32", (P, P), f32)
    inp("inv40", (P, cfg.OUT_F), f32)
    inp("gsrc1", (cfg.ngrp, P, GT * TE // 16), i16)
    inp("gsrc2", (cfg.ngrp, P, GT * TE // 16), i16)
    inp("gband", (cfg.ngrp, P, cfg.SPG * cfg.KC // 16), i16)
    inp("dstcomp", (cfg.ngrp, P, GT), f16)
    inp("gsegb", (cfg.ngrp, P, GT, TE), f16)
    nb = cfg.S_pad // cfg.batch
    inp("gndlo", (nb, P, cfg.batch // 16), i16)
    inp("gndhi", (nb, P, cfg.batch // 16), i16)

    intern("hx", (cfg.Npad, cfg.HXW), f16)
    intern("asadloc", (cfg.S_pad, cfg.ADW), f16)
    intern("nd1", (cfg.nd_rows, cfg.NDW), f16)
    intern("h2x_sh", (cfg.S_pad, cfg.H2XW), f16)
    intern("h2x_full", (cfg.ncores * cfg.S_pad, cfg.H2XW), f16,
           addr_space="Shared")
    intern("nd2", (cfg.nd_rows, cfg.ND2W), f16)

    outp("logits_sh", (cfg.S_pad, cfg.OUT_F), f32)
    outp("emb_sh", (cfg.S_pad, cfg.HID), f32)
    outp("soft_sh", (cfg.S_pad, cfg.OUT_F), f32)
    outp("hard_sh", (cfg.S_pad, 1), i32)

    with tile.TileContext(nc) as tc:
        with tc.tile_pool(name="giota", bufs=1) as gp:
            io = gp.tile([P, cfg.KC], f16)
            nc.sync.dma_start(out=io[:], in_=t["iotaK16"].ap())
            t["_iotaK"] = io
            emit(tc, nc, cfg, t)
    nc.compile()
    return nc


def kernel(**inputs):
    return run(inputs, Cfg())


def run(inputs, cfg):
    edge_index = np.asarray(inputs["edge_index"])
    pp = preprocess(edge_index, cfg)
    cfg.batch = pp["batch"]
    gl = group_layout(cfg, pp)
    in_maps = build_inputs(inputs, cfg, pp, gl)
    nc = build_program(cfg)

    if os.environ.get("GAT_SIM"):
        from concourse.bass_interp import MultiCoreSim
        sim = MultiCoreSim(nc, num_cores=cfg.ncores, trace=False,
                           require_finite=False, require_nnan=False)
        cores = list(sim.cores.values())
        for k, cs in enumerate(cores):
            for name, arr in in_maps[k].items():
                cs.tensor(name)[:] = arr
        sim.simulate(check_with_hw=False)
        results = [{n: cs.tensor(n) for n in
                    ("logits_sh", "emb_sh", "soft_sh", "hard_sh")}
                   for cs in cores]
        exec_ns = None
    else:
        kwargs = {}
        if os.environ.get("GAT_TRACE"):
            kwargs = dict(trace=True, tmpdir=os.environ.get("GAT_TRACE_DIR"))
        res = run_bass_kernel_spmd(nc, in_maps, core_ids=list(range(cfg.ncores)),
                                   **kwargs)
        results = res.results
        exec_ns = res.exec_time_ns
        run.last_results = res

    logits = np.zeros((cfg.N, cfg.OUT_F), np.float32)
    emb = np.zeros((cfg.N, cfg.HID), np.float32)
    soft = np.zeros((cfg.N, cfg.OUT_F), np.float32)
    hard = np.zeros((cfg.N,), np.int32)
    for k in range(cfg.ncores):
        lo, hi = pp["cores"][k]["node_lo"], pp["cores"][k]["node_hi"]
        logits[lo:hi] = results[k]["logits_sh"][:hi - lo]
        emb[lo:hi] = results[k]["emb_sh"][:hi - lo]
        soft[lo:hi] = results[k]["soft_sh"][:hi - lo]
        hard[lo:hi] = results[k]["hard_sh"][:hi - lo, 0]
    kernel.last_exec_ns = run.last_exec_ns = exec_ns
    return logits, emb, soft, hard
